# revision 1
# baseline (speedup 1.0000x reference)
"""Trainium2 Bass kernel for 6-head causal self-attention (nn_MultiHeadAttention).

Full-input contract: kernel(**inputs) takes the unsharded numpy inputs and
returns the full [16, 2048, 384] output. Internally the batch dim (16) is
sharded 2-per-core across 8 NeuronCores (data parallel, no collectives).

Per-core pipeline (per batch):
  1. QKV projections as fp32r matmuls on x^T (pre-transposed on host).
     Q^T/K^T land head-pair-packed: partitions 0:64 = even head's d-dim,
     64:128 = odd head's, enabling K=64 row-tiled matmul pairs.
  2. Causal attention computed transposed: S^T[s, t] tiles via
     matmul(lhsT=K^T, rhs=Q^T); exp on ScalarE with fused 1/8 scale
     (scores are O(1), so softmax needs no max subtraction); causal zeroing
     via gpsimd affine_select on diagonal tiles only; U^T = V^T @ P^T via
     matmul(lhsT=[V_h | ones64], rhs=P^T) which also produces the softmax
     row-sums replicated on partitions 64:128 for free.
  3. Normalization: 1/r = exp(-ln r) on ScalarE (ln+exp share one ACT
     table set), multiply on VectorE.
  4. Output projection + bias, written transposed; host undoes transposes.
"""

import sys

for _p in ("/opt/trn_rl_repo",):
    if _p not in sys.path:
        sys.path.insert(0, _p)

import numpy as np

B, T, C = 16, 2048, 384
H, DH = 6, 64
NCORES = 8
BPC = B // NCORES  # batches per core
KC = C // 128      # 3 contraction chunks
NTQ = T // 512     # 4 query blocks
NSI = T // 128     # 16 key tiles

_CACHE = {}


def _build():
    if "nc" in _CACHE:
        return _CACHE["nc"]

    import bass_rust as _bass_rust
    import concourse.bacc as bacc
    import concourse.mybir as mybir
    import concourse.tile as tile
    from concourse.hw_specs import get_activation_tables

    dt = mybir.dt
    AF = mybir.ActivationFunctionType
    OP = mybir.AluOpType

    class _Bacc(bacc.Bacc):
        # This kernel only uses Exp and Ln on ScalarE. Both live in the
        # natural_log_exp_and_others table set; without this filter the
        # table picker alternates between exp-only and ln+exp sets,
        # inserting an ACT_TABLE_LOAD (~1.5us) per switch.
        def insert_act_table_loads(self):
            has_activation = any(
                isinstance(i, mybir.InstActivation)
                for b in self.main_func.blocks
                for i in b.instructions
            )
            if not has_activation:
                return
            keep = {"natural_log_exp_and_others"}
            tables = [
                (n, (s if n in keep else (s - {AF.Exp, AF.Ln})))
                for n, s in get_activation_tables(self.m.arch).items()
            ]
            _bass_rust.insert_act_table_loads(self, tables)

    nc = _Bacc("TRN2", target_bir_lowering=False, debug=True)

    # Host pre-packs everything partition-major so each tensor loads with a
    # single DMA descriptor (descriptor issue is ~600ns each on the Sync
    # queue — 15 small DMAs serialized the old kernel head).
    xT_d = nc.dram_tensor("xT", [BPC, 128, KC * T], dt.bfloat16, kind="ExternalInput")
    wqk_d = nc.dram_tensor("Wqk", [128, KC * 768], dt.bfloat16, kind="ExternalInput")
    wv_d = nc.dram_tensor("Wv", [128, KC * 384], dt.bfloat16, kind="ExternalInput")
    wo_d = nc.dram_tensor("Wo", [128, KC * 384], dt.bfloat16, kind="ExternalInput")
    bo_d = nc.dram_tensor("bo", [128, KC], dt.float32, kind="ExternalInput")
    yT_d = nc.dram_tensor("yT", [BPC, KC, 128, T], dt.float32, kind="ExternalOutput")

    with tile.TileContext(nc) as tc:
        with (
            tc.tile_pool(name="wp", bufs=1) as wp,
            tc.tile_pool(name="vp", bufs=2) as vp,
            tc.tile_pool(name="xp", bufs=2) as xp,
            tc.tile_pool(name="pp", bufs=6) as pp,
            tc.tile_pool(name="np_", bufs=4) as np_,
            tc.tile_pool(name="yp", bufs=2) as yp,
            tc.tile_pool(name="ups", bufs=2, space="PSUM") as ups,
            tc.tile_pool(name="mm", bufs=2, space="PSUM") as mm,
            tc.tile_pool(name="sp", bufs=2, space="PSUM") as sp,
        ):
            # ---- constants (one DMA each; wqk + x first so the first
            # projection matmul can start as early as possible) ----
            wqk = wp.tile([128, KC, 768], dt.bfloat16, name="wqk")
            wv = wp.tile([128, KC, 384], dt.bfloat16, name="wv")
            wo = wp.tile([128, KC, 384], dt.bfloat16, name="wo")
            bo = wp.tile([128, KC], dt.float32, name="bo")

            # Deferred PE work queue: projection matmuls are drained one
            # unit per attention si-step so the (ACT-bound) attention loop
            # hides the (PE-only) projection phases.
            fillers = []

            def drain(n=1):
                for _ in range(n):
                    if fillers:
                        fillers.pop(0)()

            def flush():
                while fillers:
                    fillers.pop(0)()

            def load_x(b):
                xt = xp.tile([128, KC, T], dt.bfloat16, name="xt")
                nc.sync.dma_start(xt[:], xT_d[b])
                return xt

            def new_vones():
                # V with ones columns per head: [s % 128, si, h, e|ones]
                vones = vp.tile([128, NSI, H, 128], dt.bfloat16, name="vones")
                nc.gpsimd.memset(vones[:, :, :, 64:128], 1.0)
                return vones

            def v_unit(xt, vones, ti):
                def emit():
                    ps = mm.tile([128, 512], dt.float32, name="ps_mm")
                    for k in range(KC):
                        nc.tensor.matmul(
                            ps[:, 0:384],
                            xt[:, k, 128 * ti : 128 * ti + 128],
                            wv[:, k, :],
                            start=(k == 0),
                            stop=(k == KC - 1),
                        )
                    nc.vector.tensor_copy(
                        out=vones[:, ti, :, 0:64], in_=ps[:, 0:384]
                    )
                return emit

            def qk_unit(xt, qt, kt, p, tq, qk):
                def emit():
                    ps = mm.tile([128, 512], dt.float32, name="ps_mm")
                    for k in range(KC):
                        nc.tensor.matmul(
                            ps[:],
                            wqk[:, k, 256 * p + 128 * qk : 256 * p + 128 * qk + 128],
                            xt[:, k, 512 * tq : 512 * tq + 512],
                            start=(k == 0),
                            stop=(k == KC - 1),
                        )
                    dst = qt if qk == 0 else kt
                    nc.vector.tensor_copy(
                        out=dst[:, p, 512 * tq : 512 * tq + 512], in_=ps[:]
                    )
                return emit

            def oproj_unit(b, ot, tq, mo):
                def emit():
                    ps = mm.tile([128, 512], dt.float32, name="ps_mm")
                    for k in range(KC):
                        nc.tensor.matmul(
                            ps[:],
                            wo[:, k, 128 * mo : 128 * mo + 128],
                            ot[:, k, 512 * tq : 512 * tq + 512],
                            start=(k == 0),
                            stop=(k == KC - 1),
                        )
                    yt = yp.tile([128, 512], dt.float32, name="yt")
                    nc.vector.tensor_tensor(
                        out=yt[:],
                        in0=ps[:],
                        in1=bo[:, mo, None].to_broadcast([128, 512]),
                        op=OP.add,
                    )
                    nc.sync.dma_start(
                        yT_d[b, mo, :, 512 * tq : 512 * tq + 512], yt[:]
                    )
                return emit

            def attention_pair(qt, kt, vones, ot, p, qb_hook=None):
                for qb in range(NTQ):
                    u0 = ups.tile([128, 512], dt.float32, name="ps_u")
                    u1 = ups.tile([128, 512], dt.float32, name="ps_u")
                    nsi = 4 * qb + 4
                    pts = {}

                    def emit_u(si, nsi=nsi, u0=u0, u1=u1, pts=pts, qb=qb):
                        pt = pts.pop(si)
                        # columns 0:lo of this si's P tile are fully masked
                        # (zero attention) — skip them instead of zeroing
                        lo = 128 * (si - 4 * qb) if si >= 4 * qb else 0
                        for hf, uu in ((0, u0), (1, u1)):
                            nc.tensor.matmul(
                                uu[:, lo:],
                                vones[:, si, 2 * p + hf, :],
                                pt[:, hf, lo:],
                                start=(si == 0),
                                stop=(si == nsi - 1),
                            )

                    for si in range(nsi):
                        diag = si >= 4 * qb
                        d = si - 4 * qb if diag else 0
                        lo = 128 * d  # fully-masked columns to skip
                        sps = sp.tile([128, 1024], dt.float32, name="sps")
                        spv = sps[:].rearrange("p (h t) -> p h t", h=2)
                        for hf in range(2):
                            nc.tensor.matmul(
                                spv[:, hf, lo:512],
                                kt[64 * hf : 64 * hf + 64, p,
                                   128 * si : 128 * si + 128],
                                qt[64 * hf : 64 * hf + 64, p,
                                   512 * qb + lo : 512 * qb + 512],
                                start=True,
                                stop=True,
                            )
                        pt = pp.tile([128, 2, 512], dt.bfloat16, name="pt")
                        nc.scalar.activation(
                            pt[:, :, lo:], spv[:, :, lo:], AF.Exp, scale=0.125
                        )
                        if diag:
                            # zero the still-masked triangle in the 128-col
                            # diagonal window: keep iff f >= p
                            nc.gpsimd.affine_select(
                                out=pt[:, :, lo : lo + 128],
                                in_=pt[:, :, lo : lo + 128],
                                compare_op=OP.is_ge,
                                fill=0.0,
                                base=0,
                                channel_multiplier=-1,
                                pattern=[[0, 2], [1, 128]],
                            )
                        pts[si] = pt
                        # software pipeline: U-matmuls three si behind the
                        # S-matmuls so exp(si)+affine(si) (the diagonal
                        # mask adds a GpSimd hop) overlap S on PE without
                        # blocking the in-order PE queue
                        if si > 2:
                            emit_u(si - 3)
                        drain(1)
                    for k in range(3, 0, -1):
                        emit_u(nsi - k)
                    for hh, uu in ((2 * p, u0), (2 * p + 1, u1)):
                        # Normalization entirely on DVE (keeps the ACT queue
                        # free for the attention exp stream): shift-copy the
                        # replicated row-sums r down to base partition 0,
                        # 1/r via one custom DVE op (base 0 only — custom-DVE
                        # ops misbehave at base_partition != 0), then one
                        # mixed PSUM/SBUF multiply straight out of the U psum.
                        rcp = np_.tile([64, 512], dt.float32, name="rcp")
                        nc.vector.tensor_copy(out=rcp[:], in_=uu[64:128, :])
                        rec = np_.tile([64, 512], dt.float32, name="rec")
                        nc.vector.reciprocal_approx_fast(out=rec[:], in_=rcp[:])
                        nc.vector.tensor_tensor(
                            out=ot[64 * (hh % 2) : 64 * (hh % 2) + 64, p,
                                   512 * qb : 512 * qb + 512],
                            in0=uu[0:64, :],
                            in1=rec[:],
                            op=OP.mult,
                        )
                    if qb_hook is not None:
                        qb_hook(qb)

            # First batch: interleave wqk/x chunk DMAs so the first
            # projection matmul (k=0) starts after ~1/3 of the data lands.
            xt = xp.tile([128, KC, T], dt.bfloat16, name="xt")
            for k in range(KC):
                nc.sync.dma_start(wqk[:, k], wqk_d[:, 768 * k : 768 * (k + 1)])
                nc.sync.dma_start(xt[:, k], xT_d[0, :, T * k : T * (k + 1)])
            nc.sync.dma_start(wv[:], wv_d[:])
            nc.sync.dma_start(bo[:], bo_d[:])
            nc.sync.dma_start(wo[:], wo_d[:])
            vones = new_vones()
            qt = xp.tile([128, 3, T], dt.bfloat16, name="qt")
            kt = xp.tile([128, 3, T], dt.bfloat16, name="kt")
            for tq in range(NTQ):
                for qk in range(2):
                    qk_unit(xt, qt, kt, 0, tq, qk)()
            for ti in range(NSI):
                fillers.append(v_unit(xt, vones, ti))

            prev = None  # (b, ot) with pending out-proj
            for b in range(BPC):
                ot = xp.tile([128, 3, T], dt.bfloat16, name="ot")
                for p in range(3):
                    if p < 2:
                        for tq in range(NTQ):
                            for qk in range(2):
                                fillers.append(qk_unit(xt, qt, kt, p + 1, tq, qk))
                    elif b + 1 < BPC:
                        nxt = load_x(b + 1)
                        nvones = new_vones()
                        nqt = xp.tile([128, 3, T], dt.bfloat16, name="qt")
                        nkt = xp.tile([128, 3, T], dt.bfloat16, name="kt")
                        for ti in range(NSI):
                            fillers.append(v_unit(nxt, nvones, ti))
                        for tq in range(NTQ):
                            for qk in range(2):
                                fillers.append(qk_unit(nxt, nqt, nkt, 0, tq, qk))
                    attention_pair(qt, kt, vones, ot, p)
                if prev is not None:
                    pb, pot = prev
                    for tq in range(NTQ):
                        for mo in range(KC):
                            fillers.append(oproj_unit(pb, pot, tq, mo))
                prev = (b, ot)
                if b + 1 < BPC:
                    xt, vones, qt, kt = nxt, nvones, nqt, nkt
            flush()
            pb, pot = prev
            for tq in range(NTQ):
                for mo in range(KC):
                    oproj_unit(pb, pot, tq, mo)()

    nc.compile()
    _CACHE["nc"] = nc
    return nc


def _prep_inputs(x, Wq, Wk, Wv, Wo, bo):
    import ml_dtypes
    bf16 = ml_dtypes.bfloat16
    x = np.ascontiguousarray(np.asarray(x, dtype=np.float32))
    Wq = np.asarray(Wq, dtype=np.float32)
    Wk = np.asarray(Wk, dtype=np.float32)
    Wv = np.asarray(Wv, dtype=np.float32)
    Wo = np.asarray(Wo, dtype=np.float32)
    bo = np.asarray(bo, dtype=np.float32)

    # All tensors packed partition-major so the kernel loads each with one
    # DMA: element [pc, k*cols + j] = chunk k, partition pc, column j.
    # x^T: [B, T, C] -> [B, C, T] -> [B, 128, KC*T]
    xT = np.ascontiguousarray(
        x.transpose(0, 2, 1).reshape(B, KC, 128, T).transpose(0, 2, 1, 3)
        .reshape(B, 128, KC * T)
    ).astype(bf16)

    # Wqk columns per pair p: [Q_2p | Q_2p+1 | K_2p | K_2p+1], 64 each
    wqk = np.empty((C, 768), np.float32)
    for p in range(3):
        wqk[:, 256 * p + 0 : 256 * p + 64] = Wq[2 * p]
        wqk[:, 256 * p + 64 : 256 * p + 128] = Wq[2 * p + 1]
        wqk[:, 256 * p + 128 : 256 * p + 192] = Wk[2 * p]
        wqk[:, 256 * p + 192 : 256 * p + 256] = Wk[2 * p + 1]
    wqk = np.ascontiguousarray(
        wqk.reshape(KC, 128, 768).transpose(1, 0, 2).reshape(128, KC * 768)
    ).astype(bf16)

    # Wv columns (h*64+e), rows C
    wv = np.ascontiguousarray(
        Wv.transpose(1, 0, 2).reshape(KC, 128, H * DH).transpose(1, 0, 2)
        .reshape(128, KC * H * DH)
    ).astype(bf16)
    wo = np.ascontiguousarray(
        Wo.reshape(KC, 128, C).transpose(1, 0, 2).reshape(128, KC * C)
    ).astype(bf16)
    bo_r = np.ascontiguousarray(bo.reshape(KC, 128).T)
    return xT, wqk, wv, wo, bo_r


def _run(inputs, trace=False):
    from concourse.bass_utils import run_bass_kernel_spmd

    nc = _build()
    xT, wqk, wv, wo, bo_r = _prep_inputs(**inputs)
    in_maps = [
        {
            "xT": xT[BPC * i : BPC * (i + 1)],
            "Wqk": wqk,
            "Wv": wv,
            "Wo": wo,
            "bo": bo_r,
        }
        for i in range(NCORES)
    ]
    res = run_bass_kernel_spmd(nc, in_maps, list(range(NCORES)), trace=trace)
    # yT per core: [BPC, KC, 128, T] -> full y [B, T, C]
    yT = np.concatenate([np.asarray(res.results[i]["yT"]) for i in range(NCORES)], axis=0)
    y = yT.reshape(B, C, T).transpose(0, 2, 1)
    return np.ascontiguousarray(y.astype(np.float32)), res.exec_time_ns


def kernel(**inputs) -> np.ndarray:
    y, _ = _run(inputs, trace=False)
    return y



# revision 2
# speedup vs baseline: 1.0462x; 1.0462x over previous
"""Trainium2 Bass kernel for 6-head causal self-attention — v2.

Data-parallel over batch (2 per core x 8 cores), full-input contract.

v2 changes vs baseline:
  - Exp offload: a fraction of si-steps compute P = quad(s) ~= exp(s/8) via
    DVE affine (psum->sbuf) + GpSimd square + GpSimd affine-to-bf16, freeing
    the ACT engine (the former co-bottleneck). Quadratic minimax on
    s/8 in [-0.95, 0.95], max rel 3.4%, softmax-renormalized -> harmless at
    ~20% share.
  - Optional SPLIT_MM: every 128-contraction matmul (QKV/O projections, U)
    issues as two concurrent 64-row-tile halves (partitions 0:64 / 64:128)
    accumulating into the same PSUM bank; with the S-pair already row-tiled
    by head, the whole kernel stays in 64x128 PE tiling mode.
  - Normalization: row-sums for both heads land in one [128,2,512] PSUM
    tile; one copy + one reciprocal per (pair, qb) instead of two of each.
"""

import sys

for _p in ("/opt/trn_rl_repo",):
    if _p not in sys.path:
        sys.path.insert(0, _p)

import numpy as np

B, T, C = 16, 2048, 384
H, DH = 6, 64
NCORES = 8
BPC = B // NCORES
KC = C // 128
NTQ = T // 512
NSI = T // 128

SPLIT_MM = False  # two concurrent 64-row-tile halves per 128-contraction matmul
QUAD_MOD = 10**9  # si-steps with global_idx % QUAD_MOD == QUAD_PHASE use quad-exp
QUAD_PHASE = 2
# p ~= (Cv*s + Dv)^2 + Eq  (s = raw score; fits exp(s/8) on s/8 in [-.95,.95])
QCV = 0.08590377261863319
QDV = 0.8030737083603579
QE = 0.37740867433016057

_CACHE = {}


def _build():
    key = "nc"
    if key in _CACHE:
        return _CACHE[key]

    import bass_rust as _bass_rust
    import concourse.bacc as bacc
    import concourse.mybir as mybir
    import concourse.tile as tile
    from concourse.hw_specs import get_activation_tables

    dt = mybir.dt
    AF = mybir.ActivationFunctionType
    OP = mybir.AluOpType

    class _Bacc(bacc.Bacc):
        # Only Exp is used on ScalarE; keep one table set resident so the
        # picker never inserts a mid-kernel ACT_TABLE_LOAD (~1.5us).
        def insert_act_table_loads(self):
            has_activation = any(
                isinstance(i, mybir.InstActivation)
                for b in self.main_func.blocks
                for i in b.instructions
            )
            if not has_activation:
                return
            keep = {"natural_log_exp_and_others"}
            tables = [
                (n, (s if n in keep else (s - {AF.Exp, AF.Ln})))
                for n, s in get_activation_tables(self.m.arch).items()
            ]
            _bass_rust.insert_act_table_loads(self, tables)

    nc = _Bacc("TRN2", target_bir_lowering=False, debug=True)

    xT_d = nc.dram_tensor("xT", [BPC, 128, KC * T], dt.bfloat16, kind="ExternalInput")
    wqk_d = nc.dram_tensor("Wqk", [128, KC * 768], dt.bfloat16, kind="ExternalInput")
    wv_d = nc.dram_tensor("Wv", [128, KC * 384], dt.bfloat16, kind="ExternalInput")
    wo_d = nc.dram_tensor("Wo", [128, KC * 384], dt.bfloat16, kind="ExternalInput")
    bo_d = nc.dram_tensor("bo", [128, KC], dt.float32, kind="ExternalInput")
    yT_d = nc.dram_tensor("yT", [BPC, KC, 128, T], dt.float32, kind="ExternalOutput")

    with tile.TileContext(nc) as tc:
        with (
            tc.tile_pool(name="wp", bufs=1) as wp,
            tc.tile_pool(name="vp", bufs=2) as vp,
            tc.tile_pool(name="xp", bufs=2) as xp,
            tc.tile_pool(name="pp", bufs=10) as pp,
            tc.tile_pool(name="qp", bufs=2) as qp,
            tc.tile_pool(name="np_", bufs=2) as np_,
            tc.tile_pool(name="yp", bufs=2) as yp,
            tc.tile_pool(name="ups", bufs=1, space="PSUM") as ups,
            tc.tile_pool(name="mm", bufs=2, space="PSUM") as mm,
            tc.tile_pool(name="sp", bufs=2, space="PSUM") as sp,
        ):
            wqk = wp.tile([128, KC, 768], dt.bfloat16, name="wqk")
            wv = wp.tile([128, KC, 384], dt.bfloat16, name="wv")
            wo = wp.tile([128, KC, 384], dt.bfloat16, name="wo")
            bo = wp.tile([128, KC], dt.float32, name="bo")

            fillers = []

            def drain(n=1):
                for _ in range(n):
                    if fillers:
                        fillers.pop(0)()

            def flush():
                while fillers:
                    fillers.pop(0)()

            def msplit(ps, lhsT_lo, lhsT_hi, rhs_lo, rhs_hi, start, stop):
                # 128-contraction matmul as two concurrent 64-row-tile halves
                nc.tensor.matmul(ps, lhsT_lo, rhs_lo, start=start, stop=False)
                nc.tensor.matmul(ps, lhsT_hi, rhs_hi, start=False, stop=stop)

            def load_x(b):
                xt = xp.tile([128, KC, T], dt.bfloat16, name="xt")
                nc.sync.dma_start(xt[:], xT_d[b])
                return xt

            def new_vones():
                # [s%128, si, h, ones(64) | e(64)] — sums land on psum parts
                # 0:64 (base 0, so reciprocal can read them in place)
                vones = vp.tile([128, NSI, H, 128], dt.bfloat16, name="vones")
                nc.gpsimd.memset(vones[:, :, :, 0:64], 1.0)
                return vones

            def v_unit(xt, vones, ti):
                def emit():
                    ps = mm.tile([128, 512], dt.float32, name="ps_mm")
                    for k in range(KC):
                        if SPLIT_MM:
                            msplit(
                                ps[:, 0:384],
                                xt[0:64, k, 128 * ti : 128 * ti + 128],
                                xt[64:128, k, 128 * ti : 128 * ti + 128],
                                wv[0:64, k, :],
                                wv[64:128, k, :],
                                start=(k == 0),
                                stop=(k == KC - 1),
                            )
                        else:
                            nc.tensor.matmul(
                                ps[:, 0:384],
                                xt[:, k, 128 * ti : 128 * ti + 128],
                                wv[:, k, :],
                                start=(k == 0),
                                stop=(k == KC - 1),
                            )
                    nc.vector.tensor_copy(
                        out=vones[:, ti, :, 64:128], in_=ps[:, 0:384]
                    )
                return emit

            def qk_unit(xt, qt, kt, p, tq, qk):
                def emit():
                    ps = mm.tile([128, 512], dt.float32, name="ps_mm")
                    cs = 256 * p + 128 * qk
                    for k in range(KC):
                        if SPLIT_MM:
                            msplit(
                                ps[:],
                                wqk[0:64, k, cs : cs + 128],
                                wqk[64:128, k, cs : cs + 128],
                                xt[0:64, k, 512 * tq : 512 * tq + 512],
                                xt[64:128, k, 512 * tq : 512 * tq + 512],
                                start=(k == 0),
                                stop=(k == KC - 1),
                            )
                        else:
                            nc.tensor.matmul(
                                ps[:],
                                wqk[:, k, cs : cs + 128],
                                xt[:, k, 512 * tq : 512 * tq + 512],
                                start=(k == 0),
                                stop=(k == KC - 1),
                            )
                    dst = qt if qk == 0 else kt
                    nc.vector.tensor_copy(
                        out=dst[:, p, 512 * tq : 512 * tq + 512], in_=ps[:]
                    )
                return emit

            def oproj_unit(b, ot, tq, mo):
                def emit():
                    ps = mm.tile([128, 512], dt.float32, name="ps_mm")
                    for k in range(KC):
                        if SPLIT_MM:
                            msplit(
                                ps[:],
                                wo[0:64, k, 128 * mo : 128 * mo + 128],
                                wo[64:128, k, 128 * mo : 128 * mo + 128],
                                ot[0:64, k, 512 * tq : 512 * tq + 512],
                                ot[64:128, k, 512 * tq : 512 * tq + 512],
                                start=(k == 0),
                                stop=(k == KC - 1),
                            )
                        else:
                            nc.tensor.matmul(
                                ps[:],
                                wo[:, k, 128 * mo : 128 * mo + 128],
                                ot[:, k, 512 * tq : 512 * tq + 512],
                                start=(k == 0),
                                stop=(k == KC - 1),
                            )
                    yt = yp.tile([128, 512], dt.float32, name="yt")
                    nc.vector.tensor_tensor(
                        out=yt[:],
                        in0=ps[:],
                        in1=bo[:, mo, None].to_broadcast([128, 512]),
                        op=OP.add,
                    )
                    nc.sync.dma_start(
                        yT_d[b, mo, :, 512 * tq : 512 * tq + 512], yt[:]
                    )
                return emit

            step_counter = [0]

            def attention_pair(qt, kt, vones, ot, p):
                for qb in range(NTQ):
                    uu = ups.tile([128, 2, 512], dt.float32, name="ps_u")
                    nsi = 4 * qb + 4
                    pts = {}

                    def emit_u(si, start, stop, uu=uu, pts=pts, qb=qb):
                        pt = pts.pop(si)
                        lo = 128 * (si - 4 * qb) if si >= 4 * qb else 0
                        for hf in range(2):
                            h = 2 * p + hf
                            if SPLIT_MM:
                                nc.tensor.matmul(
                                    uu[:, hf, lo:],
                                    vones[0:64, si, h, :],
                                    pt[0:64, 512 * hf + lo : 512 * hf + 512],
                                    start=start,
                                    stop=False,
                                )
                                nc.tensor.matmul(
                                    uu[:, hf, lo:],
                                    vones[64:128, si, h, :],
                                    pt[64:128, 512 * hf + lo : 512 * hf + 512],
                                    start=False,
                                    stop=stop,
                                )
                            else:
                                nc.tensor.matmul(
                                    uu[:, hf, lo:],
                                    vones[:, si, h, :],
                                    pt[:, 512 * hf + lo : 512 * hf + 512],
                                    start=start,
                                    stop=stop,
                                )

                    def s_step(si):
                        diag = si >= 4 * qb
                        d = si - 4 * qb if diag else 0
                        lo = 128 * d
                        sps = sp.tile([128, 1024], dt.float32, name="sps")
                        spv = sps[:].rearrange("p (h t) -> p h t", h=2)
                        for hf in range(2):
                            nc.tensor.matmul(
                                spv[:, hf, lo:512],
                                kt[64 * hf : 64 * hf + 64, p,
                                   128 * si : 128 * si + 128],
                                qt[64 * hf : 64 * hf + 64, p,
                                   512 * qb + lo : 512 * qb + 512],
                                start=True,
                                stop=True,
                            )
                        return sps, spv, lo, diag

                    def exp_step(si, sps, spv, lo, diag):
                        pt = pp.tile([128, 1024], dt.bfloat16, name="pt")
                        ptv = pt[:].rearrange("p (h t) -> p h t", h=2)
                        use_quad = (step_counter[0] % QUAD_MOD) == QUAD_PHASE
                        step_counter[0] += 1
                        if use_quad:
                            # full-tile contiguous APs: GpSimd strided ops are
                            # ~10x slower; masked cols compute garbage that the
                            # U matmuls never read (lo-skip + affine_select)
                            vt = qp.tile([128, 1024], dt.float32, name="vt")
                            nc.vector.tensor_scalar(
                                out=vt[:], in0=sps[:],
                                scalar1=QCV, scalar2=QDV,
                                op0=OP.mult, op1=OP.add,
                            )
                            wt = qp.tile([128, 1024], dt.float32, name="wt")
                            nc.gpsimd.tensor_tensor(
                                out=wt[:], in0=vt[:], in1=vt[:], op=OP.mult,
                            )
                            nc.gpsimd.tensor_scalar(
                                out=pt[:], in0=wt[:],
                                scalar1=1.0, scalar2=QE, op0=OP.mult, op1=OP.add,
                            )
                        else:
                            nc.scalar.activation(
                                ptv[:, :, lo:], spv[:, :, lo:], AF.Exp, scale=0.125
                            )
                        if diag:
                            nc.gpsimd.affine_select(
                                out=ptv[:, :, lo : lo + 128],
                                in_=ptv[:, :, lo : lo + 128],
                                compare_op=OP.is_ge,
                                fill=0.0,
                                base=0,
                                channel_multiplier=-1,
                                pattern=[[0, 2], [1, 128]],
                            )
                        pts[si] = pt

                    # single-step pipeline. U accumulation order is free (it's a
                    # sum), so slow-producer (quad-chain) steps defer their U to
                    # the qb tail — the 3-engine chain never gates the pipeline.
                    # ACT-produced steps emit U LAG steps later.
                    LAG = 5
                    fast_q = []   # ACT-produced steps awaiting U emission
                    defer = []    # quad-produced steps, emitted at qb end
                    n_emitted = [0]
                    n_total = 2 * nsi  # emissions counted per (hf)

                    def emit_next(si):
                        first = n_emitted[0] == 0
                        n_emitted[0] += 1
                        last = n_emitted[0] == nsi
                        emit_u(si, start=first, stop=last)

                    for si in range(nsi):
                        was_quad = (step_counter[0] % QUAD_MOD) == QUAD_PHASE
                        exp_step(si, *s_step(si))
                        (defer if was_quad else fast_q).append(si)
                        if len(fast_q) > LAG or (fast_q and si == nsi - 1):
                            emit_next(fast_q.pop(0))
                        drain(1)
                    for si in fast_q:
                        emit_next(si)
                    for si in defer:
                        emit_next(si)
                    # normalization: sums for both heads at psum parts 0:64;
                    # reciprocal reads the PSUM sums in place (base 0)
                    rec = np_.tile([64, 2, 512], dt.float32, name="rec")
                    nc.vector.reciprocal_approx_fast(out=rec[:], in_=uu[0:64, :, :])
                    for hf in range(2):
                        hh = 2 * p + hf
                        nc.vector.tensor_tensor(
                            out=ot[64 * (hh % 2) : 64 * (hh % 2) + 64, p,
                                   512 * qb : 512 * qb + 512],
                            in0=uu[64:128, hf, :],
                            in1=rec[:, hf, :],
                            op=OP.mult,
                        )

            # ---- constants + first batch ----
            xt = xp.tile([128, KC, T], dt.bfloat16, name="xt")
            for k in range(KC):
                nc.sync.dma_start(wqk[:, k], wqk_d[:, 768 * k : 768 * (k + 1)])
                nc.sync.dma_start(xt[:, k], xT_d[0, :, T * k : T * (k + 1)])
            nc.sync.dma_start(wv[:], wv_d[:])
            nc.sync.dma_start(bo[:], bo_d[:])
            nc.sync.dma_start(wo[:], wo_d[:])
            vones = new_vones()
            qt = xp.tile([128, 3, T], dt.bfloat16, name="qt")
            kt = xp.tile([128, 3, T], dt.bfloat16, name="kt")
            for tq in range(NTQ):
                for qk in range(2):
                    qk_unit(xt, qt, kt, 0, tq, qk)()
            for ti in range(NSI):
                fillers.append(v_unit(xt, vones, ti))

            prev = None
            for b in range(BPC):
                ot = xp.tile([128, 3, T], dt.bfloat16, name="ot")
                for p in range(3):
                    if p < 2:
                        for tq in range(NTQ):
                            for qk in range(2):
                                fillers.append(qk_unit(xt, qt, kt, p + 1, tq, qk))
                    elif b + 1 < BPC:
                        nxt = load_x(b + 1)
                        nvones = new_vones()
                        nqt = xp.tile([128, 3, T], dt.bfloat16, name="qt")
                        nkt = xp.tile([128, 3, T], dt.bfloat16, name="kt")
                        for ti in range(NSI):
                            fillers.append(v_unit(nxt, nvones, ti))
                        for tq in range(NTQ):
                            for qk in range(2):
                                fillers.append(qk_unit(nxt, nqt, nkt, 0, tq, qk))
                    attention_pair(qt, kt, vones, ot, p)
                if prev is not None:
                    pb, pot = prev
                    for tq in range(NTQ):
                        for mo in range(KC):
                            fillers.append(oproj_unit(pb, pot, tq, mo))
                prev = (b, ot)
                if b + 1 < BPC:
                    xt, vones, qt, kt = nxt, nvones, nqt, nkt
            flush()
            pb, pot = prev
            for tq in range(NTQ):
                for mo in range(KC):
                    oproj_unit(pb, pot, tq, mo)()

    nc.compile()
    _CACHE[key] = nc
    return nc


def _prep_inputs(x, Wq, Wk, Wv, Wo, bo):
    import ml_dtypes
    bf16 = ml_dtypes.bfloat16
    x = np.ascontiguousarray(np.asarray(x, dtype=np.float32))
    Wq = np.asarray(Wq, dtype=np.float32)
    Wk = np.asarray(Wk, dtype=np.float32)
    Wv = np.asarray(Wv, dtype=np.float32)
    Wo = np.asarray(Wo, dtype=np.float32)
    bo = np.asarray(bo, dtype=np.float32)

    xT = np.ascontiguousarray(
        x.transpose(0, 2, 1).reshape(B, KC, 128, T).transpose(0, 2, 1, 3)
        .reshape(B, 128, KC * T)
    ).astype(bf16)

    wqk = np.empty((C, 768), np.float32)
    for p in range(3):
        wqk[:, 256 * p + 0 : 256 * p + 64] = Wq[2 * p]
        wqk[:, 256 * p + 64 : 256 * p + 128] = Wq[2 * p + 1]
        wqk[:, 256 * p + 128 : 256 * p + 192] = Wk[2 * p]
        wqk[:, 256 * p + 192 : 256 * p + 256] = Wk[2 * p + 1]
    wqk = np.ascontiguousarray(
        wqk.reshape(KC, 128, 768).transpose(1, 0, 2).reshape(128, KC * 768)
    ).astype(bf16)

    wv = np.ascontiguousarray(
        Wv.transpose(1, 0, 2).reshape(KC, 128, H * DH).transpose(1, 0, 2)
        .reshape(128, KC * H * DH)
    ).astype(bf16)
    wo = np.ascontiguousarray(
        Wo.reshape(KC, 128, C).transpose(1, 0, 2).reshape(128, KC * C)
    ).astype(bf16)
    bo_r = np.ascontiguousarray(bo.reshape(KC, 128).T)
    return xT, wqk, wv, wo, bo_r


def _run(inputs, trace=False):
    from concourse.bass_utils import run_bass_kernel_spmd

    nc = _build()
    xT, wqk, wv, wo, bo_r = _prep_inputs(**inputs)
    in_maps = [
        {
            "xT": xT[BPC * i : BPC * (i + 1)],
            "Wqk": wqk,
            "Wv": wv,
            "Wo": wo,
            "bo": bo_r,
        }
        for i in range(NCORES)
    ]
    res = run_bass_kernel_spmd(nc, in_maps, list(range(NCORES)), trace=trace)
    yT = np.concatenate([np.asarray(res.results[i]["yT"]) for i in range(NCORES)], axis=0)
    y = yT.reshape(B, C, T).transpose(0, 2, 1)
    return np.ascontiguousarray(y.astype(np.float32)), res.exec_time_ns


def kernel(**inputs) -> np.ndarray:
    y, _ = _run(inputs, trace=False)
    return y


# revision 3
# speedup vs baseline: 1.0610x; 1.0142x over previous
"""Trainium2 Bass kernel for 6-head causal self-attention (nn_MultiHeadAttention).

Full-input contract: kernel(**inputs) takes unsharded numpy inputs, returns
the full [16, 2048, 384] output. Batch dim (16) sharded 2-per-core across 8
NeuronCores (data parallel, no collectives).

Per-core pipeline (per batch), vs the earlier 307us version:
  - S-pairs per si-step run as two concurrent 64-row-tile matmuls (row_grp
    h0/h64) into separate PSUM banks.
  - exp on ScalarE per si-step over both heads ([128, 2, 512-lo], fused 1/8
    scale); causal zeroing via gpsimd affine_select on diagonal tiles only.
  - U^T = V^T @ P^T with ones-columns FIRST in the stationary so both heads'
    softmax row-sums land on PSUM partitions 0:64 of one [128,2,512] tile:
    one in-place reciprocal_approx_fast (PSUM src) + two tensor_tensor mults
    per (pair, qb) normalize and write ot.
  - U-accumulation order is free (it is a sum): ACT-produced steps emit U
    five steps late, and each qb's tail U-matmuls + normalization spill into
    the following qb/pair/batch's steps ("pend" queue) so the PE never
    bursts at a boundary while ScalarE starves. This removed ~14us of
    boundary stalls (303.8 -> 290.4us).
  - Projection matmuls (QKV, out-proj) drain one unit per si-step from a
    deferred queue to fill PE gaps; out-proj adds bias via the psum->sbuf
    move and DMAs y^T straight out.
  - A dormant DVE+GpSimd quadratic-exp offload path exists (QUAD_MOD) but is
    disabled: at the measured cadence the PE, not ScalarE, is co-binding, so
    offloading exp bought nothing and its 3-engine chain latency cost more.

Measured: 290,386 ns on hardware, rel err 3.44e-3 (gate 2e-2).
"""

import sys

for _p in ("/opt/trn_rl_repo",):
    if _p not in sys.path:
        sys.path.insert(0, _p)

import numpy as np

B, T, C = 16, 2048, 384
H, DH = 6, 64
NCORES = 8
BPC = B // NCORES
KC = C // 128
NTQ = T // 512
NSI = T // 128

SPLIT_MM = False  # two concurrent 64-row-tile halves per 128-contraction matmul
QUAD_MOD = 10**9  # si-steps with global_idx % QUAD_MOD == QUAD_PHASE use quad-exp
QUAD_PHASE = 2
# p ~= (Cv*s + Dv)^2 + Eq  (s = raw score; fits exp(s/8) on s/8 in [-.95,.95])
QCV = 0.08590377261863319
QDV = 0.8030737083603579
QE = 0.37740867433016057

_CACHE = {}


def _build():
    key = "nc"
    if key in _CACHE:
        return _CACHE[key]

    import bass_rust as _bass_rust
    import concourse.bacc as bacc
    import concourse.mybir as mybir
    import concourse.tile as tile
    from concourse.hw_specs import get_activation_tables

    dt = mybir.dt
    AF = mybir.ActivationFunctionType
    OP = mybir.AluOpType

    class _Bacc(bacc.Bacc):
        # Only Exp is used on ScalarE; keep one table set resident so the
        # picker never inserts a mid-kernel ACT_TABLE_LOAD (~1.5us).
        def insert_act_table_loads(self):
            has_activation = any(
                isinstance(i, mybir.InstActivation)
                for b in self.main_func.blocks
                for i in b.instructions
            )
            if not has_activation:
                return
            keep = {"natural_log_exp_and_others"}
            tables = [
                (n, (s if n in keep else (s - {AF.Exp, AF.Ln})))
                for n, s in get_activation_tables(self.m.arch).items()
            ]
            _bass_rust.insert_act_table_loads(self, tables)

    nc = _Bacc("TRN2", target_bir_lowering=False, debug=True)

    xT_d = nc.dram_tensor("xT", [BPC, 128, KC * T], dt.bfloat16, kind="ExternalInput")
    wqk_d = nc.dram_tensor("Wqk", [128, KC * 768], dt.bfloat16, kind="ExternalInput")
    wv_d = nc.dram_tensor("Wv", [128, KC * 384], dt.bfloat16, kind="ExternalInput")
    wo_d = nc.dram_tensor("Wo", [128, KC * 384], dt.bfloat16, kind="ExternalInput")
    bo_d = nc.dram_tensor("bo", [128, KC], dt.float32, kind="ExternalInput")
    yT_d = nc.dram_tensor("yT", [BPC, KC, 128, T], dt.float32, kind="ExternalOutput")

    with tile.TileContext(nc) as tc:
        with (
            tc.tile_pool(name="wp", bufs=1) as wp,
            tc.tile_pool(name="vp", bufs=2) as vp,
            tc.tile_pool(name="xp", bufs=2) as xp,
            tc.tile_pool(name="pp", bufs=10) as pp,
            tc.tile_pool(name="qp", bufs=2) as qp,
            tc.tile_pool(name="np_", bufs=2) as np_,
            tc.tile_pool(name="yp", bufs=2) as yp,
            tc.tile_pool(name="ups", bufs=1, space="PSUM") as ups,
            tc.tile_pool(name="mm", bufs=2, space="PSUM") as mm,
            tc.tile_pool(name="sp", bufs=2, space="PSUM") as sp,
        ):
            wqk = wp.tile([128, KC, 768], dt.bfloat16, name="wqk")
            wv = wp.tile([128, KC, 384], dt.bfloat16, name="wv")
            wo = wp.tile([128, KC, 384], dt.bfloat16, name="wo")
            bo = wp.tile([128, KC], dt.float32, name="bo")

            fillers = []

            def drain(n=1):
                for _ in range(n):
                    if fillers:
                        fillers.pop(0)()

            def flush():
                while fillers:
                    fillers.pop(0)()

            def msplit(ps, lhsT_lo, lhsT_hi, rhs_lo, rhs_hi, start, stop):
                # 128-contraction matmul as two concurrent 64-row-tile halves
                nc.tensor.matmul(ps, lhsT_lo, rhs_lo, start=start, stop=False)
                nc.tensor.matmul(ps, lhsT_hi, rhs_hi, start=False, stop=stop)

            def load_x(b):
                xt = xp.tile([128, KC, T], dt.bfloat16, name="xt")
                nc.sync.dma_start(xt[:], xT_d[b])
                return xt

            def new_vones():
                # [s%128, si, h, ones(64) | e(64)] — sums land on psum parts
                # 0:64 (base 0, so reciprocal can read them in place)
                vones = vp.tile([128, NSI, H, 128], dt.bfloat16, name="vones")
                nc.gpsimd.memset(vones[:, :, :, 0:64], 1.0)
                return vones

            def v_unit(xt, vones, ti):
                def emit():
                    ps = mm.tile([128, 512], dt.float32, name="ps_mm")
                    for k in range(KC):
                        if SPLIT_MM:
                            msplit(
                                ps[:, 0:384],
                                xt[0:64, k, 128 * ti : 128 * ti + 128],
                                xt[64:128, k, 128 * ti : 128 * ti + 128],
                                wv[0:64, k, :],
                                wv[64:128, k, :],
                                start=(k == 0),
                                stop=(k == KC - 1),
                            )
                        else:
                            nc.tensor.matmul(
                                ps[:, 0:384],
                                xt[:, k, 128 * ti : 128 * ti + 128],
                                wv[:, k, :],
                                start=(k == 0),
                                stop=(k == KC - 1),
                            )
                    nc.vector.tensor_copy(
                        out=vones[:, ti, :, 64:128], in_=ps[:, 0:384]
                    )
                return emit

            def qk_unit(xt, qt, kt, p, tq, qk):
                def emit():
                    ps = mm.tile([128, 512], dt.float32, name="ps_mm")
                    cs = 256 * p + 128 * qk
                    for k in range(KC):
                        if SPLIT_MM:
                            msplit(
                                ps[:],
                                wqk[0:64, k, cs : cs + 128],
                                wqk[64:128, k, cs : cs + 128],
                                xt[0:64, k, 512 * tq : 512 * tq + 512],
                                xt[64:128, k, 512 * tq : 512 * tq + 512],
                                start=(k == 0),
                                stop=(k == KC - 1),
                            )
                        else:
                            nc.tensor.matmul(
                                ps[:],
                                wqk[:, k, cs : cs + 128],
                                xt[:, k, 512 * tq : 512 * tq + 512],
                                start=(k == 0),
                                stop=(k == KC - 1),
                            )
                    dst = qt if qk == 0 else kt
                    nc.vector.tensor_copy(
                        out=dst[:, p, 512 * tq : 512 * tq + 512], in_=ps[:]
                    )
                return emit

            def oproj_unit(b, ot, tq, mo):
                def emit():
                    ps = mm.tile([128, 512], dt.float32, name="ps_mm")
                    for k in range(KC):
                        if SPLIT_MM:
                            msplit(
                                ps[:],
                                wo[0:64, k, 128 * mo : 128 * mo + 128],
                                wo[64:128, k, 128 * mo : 128 * mo + 128],
                                ot[0:64, k, 512 * tq : 512 * tq + 512],
                                ot[64:128, k, 512 * tq : 512 * tq + 512],
                                start=(k == 0),
                                stop=(k == KC - 1),
                            )
                        else:
                            nc.tensor.matmul(
                                ps[:],
                                wo[:, k, 128 * mo : 128 * mo + 128],
                                ot[:, k, 512 * tq : 512 * tq + 512],
                                start=(k == 0),
                                stop=(k == KC - 1),
                            )
                    yt = yp.tile([128, 512], dt.float32, name="yt")
                    nc.vector.tensor_tensor(
                        out=yt[:],
                        in0=ps[:],
                        in1=bo[:, mo, None].to_broadcast([128, 512]),
                        op=OP.add,
                    )
                    nc.sync.dma_start(
                        yT_d[b, mo, :, 512 * tq : 512 * tq + 512], yt[:]
                    )
                return emit

            step_counter = [0]

            pend = []  # carried U/norm emissions, across qb/p/batch bounds

            def attention_pair(qt, kt, vones, ot, p):
                for qb in range(NTQ):
                    uu = ups.tile([128, 2, 512], dt.float32, name="ps_u")
                    nsi = 4 * qb + 4
                    pts = {}

                    def emit_u(si, start, stop, uu=uu, pts=pts, qb=qb):
                        pt = pts.pop(si)
                        lo = 128 * (si - 4 * qb) if si >= 4 * qb else 0
                        for hf in range(2):
                            h = 2 * p + hf
                            if SPLIT_MM:
                                nc.tensor.matmul(
                                    uu[:, hf, lo:],
                                    vones[0:64, si, h, :],
                                    pt[0:64, 512 * hf + lo : 512 * hf + 512],
                                    start=start,
                                    stop=False,
                                )
                                nc.tensor.matmul(
                                    uu[:, hf, lo:],
                                    vones[64:128, si, h, :],
                                    pt[64:128, 512 * hf + lo : 512 * hf + 512],
                                    start=False,
                                    stop=stop,
                                )
                            else:
                                nc.tensor.matmul(
                                    uu[:, hf, lo:],
                                    vones[:, si, h, :],
                                    pt[:, 512 * hf + lo : 512 * hf + 512],
                                    start=start,
                                    stop=stop,
                                )

                    def s_step(si):
                        diag = si >= 4 * qb
                        d = si - 4 * qb if diag else 0
                        lo = 128 * d
                        sps = sp.tile([128, 1024], dt.float32, name="sps")
                        spv = sps[:].rearrange("p (h t) -> p h t", h=2)
                        for hf in range(2):
                            nc.tensor.matmul(
                                spv[:, hf, lo:512],
                                kt[64 * hf : 64 * hf + 64, p,
                                   128 * si : 128 * si + 128],
                                qt[64 * hf : 64 * hf + 64, p,
                                   512 * qb + lo : 512 * qb + 512],
                                start=True,
                                stop=True,
                            )
                        return sps, spv, lo, diag

                    def exp_step(si, sps, spv, lo, diag):
                        pt = pp.tile([128, 1024], dt.bfloat16, name="pt")
                        ptv = pt[:].rearrange("p (h t) -> p h t", h=2)
                        use_quad = (step_counter[0] % QUAD_MOD) == QUAD_PHASE
                        step_counter[0] += 1
                        if use_quad:
                            # full-tile contiguous APs: GpSimd strided ops are
                            # ~10x slower; masked cols compute garbage that the
                            # U matmuls never read (lo-skip + affine_select)
                            vt = qp.tile([128, 1024], dt.float32, name="vt")
                            nc.vector.tensor_scalar(
                                out=vt[:], in0=sps[:],
                                scalar1=QCV, scalar2=QDV,
                                op0=OP.mult, op1=OP.add,
                            )
                            wt = qp.tile([128, 1024], dt.float32, name="wt")
                            nc.gpsimd.tensor_tensor(
                                out=wt[:], in0=vt[:], in1=vt[:], op=OP.mult,
                            )
                            nc.gpsimd.tensor_scalar(
                                out=pt[:], in0=wt[:],
                                scalar1=1.0, scalar2=QE, op0=OP.mult, op1=OP.add,
                            )
                        else:
                            nc.scalar.activation(
                                ptv[:, :, lo:], spv[:, :, lo:], AF.Exp, scale=0.125
                            )
                        if diag:
                            nc.gpsimd.affine_select(
                                out=ptv[:, :, lo : lo + 128],
                                in_=ptv[:, :, lo : lo + 128],
                                compare_op=OP.is_ge,
                                fill=0.0,
                                base=0,
                                channel_multiplier=-1,
                                pattern=[[0, 2], [1, 128]],
                            )
                        pts[si] = pt

                    # single-step pipeline. U accumulation order is free (it is
                    # a sum): ACT-produced steps emit U LAG steps later;
                    # quad-chain steps and the qb tail (incl. the norm) spill
                    # into the NEXT qb's steps so the PE never bursts at a
                    # boundary while ACT starves.
                    LAG = 5
                    fast_q = []
                    defer = []
                    n_emitted = [0]

                    def emit_next(si, nsi=nsi, emit_u=emit_u, n_emitted=n_emitted):
                        first = n_emitted[0] == 0
                        n_emitted[0] += 1
                        last = n_emitted[0] == nsi
                        emit_u(si, start=first, stop=last)

                    def norm(uu=uu, qb=qb):
                        # sums for both heads at psum parts 0:64; reciprocal
                        # reads the PSUM sums in place (base 0)
                        rec = np_.tile([64, 2, 512], dt.float32, name="rec")
                        nc.vector.reciprocal_approx_fast(out=rec[:], in_=uu[0:64, :, :])
                        for hf in range(2):
                            hh = 2 * p + hf
                            nc.vector.tensor_tensor(
                                out=ot[64 * (hh % 2) : 64 * (hh % 2) + 64, p,
                                       512 * qb : 512 * qb + 512],
                                in0=uu[64:128, hf, :],
                                in1=rec[:, hf, :],
                                op=OP.mult,
                            )

                    for si in range(nsi):
                        was_quad = (step_counter[0] % QUAD_MOD) == QUAD_PHASE
                        exp_step(si, *s_step(si))
                        (defer if was_quad else fast_q).append(si)
                        if pend:
                            pend.pop(0)()
                            if pend:
                                pend.pop(0)()
                        elif len(fast_q) > LAG:
                            emit_next(fast_q.pop(0))
                        drain(1)
                    for si in fast_q:
                        pend.append(lambda si=si, e=emit_next: e(si))
                    for si in defer:
                        pend.append(lambda si=si, e=emit_next: e(si))
                    pend.append(norm)

            # ---- constants + first batch ----
            xt = xp.tile([128, KC, T], dt.bfloat16, name="xt")
            for k in range(KC):
                nc.sync.dma_start(wqk[:, k], wqk_d[:, 768 * k : 768 * (k + 1)])
                nc.sync.dma_start(xt[:, k], xT_d[0, :, T * k : T * (k + 1)])
            nc.sync.dma_start(wv[:], wv_d[:])
            nc.sync.dma_start(bo[:], bo_d[:])
            nc.sync.dma_start(wo[:], wo_d[:])
            vones = new_vones()
            qt = xp.tile([128, 3, T], dt.bfloat16, name="qt")
            kt = xp.tile([128, 3, T], dt.bfloat16, name="kt")
            for tq in range(NTQ):
                for qk in range(2):
                    qk_unit(xt, qt, kt, 0, tq, qk)()
            for ti in range(NSI):
                fillers.append(v_unit(xt, vones, ti))

            prev = None
            for b in range(BPC):
                ot = xp.tile([128, 3, T], dt.bfloat16, name="ot")
                for p in range(3):
                    if p < 2:
                        for tq in range(NTQ):
                            for qk in range(2):
                                fillers.append(qk_unit(xt, qt, kt, p + 1, tq, qk))
                    elif b + 1 < BPC:
                        nxt = load_x(b + 1)
                        nvones = new_vones()
                        nqt = xp.tile([128, 3, T], dt.bfloat16, name="qt")
                        nkt = xp.tile([128, 3, T], dt.bfloat16, name="kt")
                        for ti in range(NSI):
                            fillers.append(v_unit(nxt, nvones, ti))
                        for tq in range(NTQ):
                            for qk in range(2):
                                fillers.append(qk_unit(nxt, nqt, nkt, 0, tq, qk))
                    attention_pair(qt, kt, vones, ot, p)
                if prev is not None:
                    pb, pot = prev
                    for tq in range(NTQ):
                        for mo in range(KC):
                            fillers.append(oproj_unit(pb, pot, tq, mo))
                prev = (b, ot)
                if b + 1 < BPC:
                    xt, vones, qt, kt = nxt, nvones, nqt, nkt
            while pend:
                pend.pop(0)()
            flush()
            pb, pot = prev
            for tq in range(NTQ):
                for mo in range(KC):
                    oproj_unit(pb, pot, tq, mo)()

    nc.compile()
    _CACHE[key] = nc
    return nc


def _prep_inputs(x, Wq, Wk, Wv, Wo, bo):
    import ml_dtypes
    bf16 = ml_dtypes.bfloat16
    x = np.ascontiguousarray(np.asarray(x, dtype=np.float32))
    Wq = np.asarray(Wq, dtype=np.float32)
    Wk = np.asarray(Wk, dtype=np.float32)
    Wv = np.asarray(Wv, dtype=np.float32)
    Wo = np.asarray(Wo, dtype=np.float32)
    bo = np.asarray(bo, dtype=np.float32)

    xT = np.ascontiguousarray(
        x.transpose(0, 2, 1).reshape(B, KC, 128, T).transpose(0, 2, 1, 3)
        .reshape(B, 128, KC * T)
    ).astype(bf16)

    wqk = np.empty((C, 768), np.float32)
    for p in range(3):
        wqk[:, 256 * p + 0 : 256 * p + 64] = Wq[2 * p]
        wqk[:, 256 * p + 64 : 256 * p + 128] = Wq[2 * p + 1]
        wqk[:, 256 * p + 128 : 256 * p + 192] = Wk[2 * p]
        wqk[:, 256 * p + 192 : 256 * p + 256] = Wk[2 * p + 1]
    wqk = np.ascontiguousarray(
        wqk.reshape(KC, 128, 768).transpose(1, 0, 2).reshape(128, KC * 768)
    ).astype(bf16)

    wv = np.ascontiguousarray(
        Wv.transpose(1, 0, 2).reshape(KC, 128, H * DH).transpose(1, 0, 2)
        .reshape(128, KC * H * DH)
    ).astype(bf16)
    wo = np.ascontiguousarray(
        Wo.reshape(KC, 128, C).transpose(1, 0, 2).reshape(128, KC * C)
    ).astype(bf16)
    bo_r = np.ascontiguousarray(bo.reshape(KC, 128).T)
    return xT, wqk, wv, wo, bo_r


def _run(inputs, trace=False):
    from concourse.bass_utils import run_bass_kernel_spmd

    nc = _build()
    xT, wqk, wv, wo, bo_r = _prep_inputs(**inputs)
    in_maps = [
        {
            "xT": xT[BPC * i : BPC * (i + 1)],
            "Wqk": wqk,
            "Wv": wv,
            "Wo": wo,
            "bo": bo_r,
        }
        for i in range(NCORES)
    ]
    res = run_bass_kernel_spmd(nc, in_maps, list(range(NCORES)), trace=trace)
    yT = np.concatenate([np.asarray(res.results[i]["yT"]) for i in range(NCORES)], axis=0)
    y = yT.reshape(B, C, T).transpose(0, 2, 1)
    return np.ascontiguousarray(y.astype(np.float32)), res.exec_time_ns


def kernel(**inputs) -> np.ndarray:
    y, _ = _run(inputs, trace=False)
    return y
